# revision 15
# baseline (speedup 1.0000x reference)
"""DCNv2 (modulated deformable convolution) on 8 Trainium2 NeuronCores.

kernel(**inputs) takes the full unsharded inputs
    x      (8, 128, 64, 64) f32
    w_om   (27, 128, 3, 3)  f32
    b_om   (27,)            f32
    weight (128, 128, 3, 3) f32
    bias   (128,)           f32
and returns the full output (8, 128, 64, 64) f32.

Sharding: pure data-parallel over batch - one image per NeuronCore, small
weights replicated; no collectives.

v8 per-core program (bf16 datapath):
  1. x/weights are cast to bf16 during the load DMA (SWDGE); x is staged
     twice into a DRAM image xt2[GROWS, 256] where row r=(y,x) holds
     [C(y,x), C(y+1,x)] - the 4 bilinear corners of any sample are 4*128
     CONTIGUOUS bf16 values (one 1KB gather descriptor per (pixel, tap)).
     Row pairs are assembled on the PE and written as 512B-contiguous
     runs, split across the sync and scalar HWDGE rings.
  2. offset conv (27ch 3x3) on the PE in bf16; sampling positions + gather
     row indices are computed before the softmax mask / bilinear
     coefficients (which overlap the first gathers); indices are wrapped
     into the 16-partition dma_gather layout with PE transposes.
  3. per (half, tap) one dma_gather (SWDGE queues round-robined 0-3 so all
     four Q7 core pairs generate descriptors in parallel) fetches
     [A0 B0 A1 B1] corner blocks in (pixel-partition, channel) layout;
     corners are combined at whole-gather granularity: ACT applies c00 per
     pixel-tile (16 ops into one [128,16,128] tile), DVE does 3
     broadcast-coefficient multiplies + 3 bf16 adds; the result is
     PE-transposed back to (channel, pixel) (4 tiles per PSUM bank) and
     accumulated over the 9 taps into PSUM with the 128x128x3x3 weight;
     bias is added on the PSUM->SBUF copy.
"""

import os
import sys

import numpy as np

sys.path.insert(0, "/opt/trn_rl_repo")

from contextlib import ExitStack

import concourse.bacc as bacc
import concourse.mybir as mybir
import concourse.tile as tile
from concourse._compat import get_trn_type
from concourse.alu_op_type import AluOpType as Alu
from concourse.bass import AP
from concourse.bass_utils import run_bass_kernel_spmd
from concourse import library_config

F32 = mybir.dt.float32
BF16 = mybir.dt.bfloat16
I32 = mybir.dt.int32
I16 = mybir.dt.int16

B = 8
C = 128
H = W = 64
HW = H * W
K2 = 9
PADG = 4
GW = H + 2 * PADG      # 72
GROWS = GW * GW        # 5184
NS = 32
NHALF = 2
SPH = NS // NHALF      # 16 s-tiles per half
PPH = HW // NHALF      # 2048 pixels per half
IDENT = mybir.ActivationFunctionType.Identity

LAST_EXEC_TIME_NS = None
LAST_RESULT = None
SINGLE_PACKET = bool(int(os.environ.get("DCN_SP", "0")))


def _emit(tc):
    nc = tc.nc
    x_d = nc.dram_tensor("x", [C, HW], F32, kind="ExternalInput").ap()
    w_om_d = nc.dram_tensor("w_om", [27, 1152], F32, kind="ExternalInput").ap()
    b_om_d = nc.dram_tensor("b_om", [27, 1], F32, kind="ExternalInput").ap()
    weight_d = nc.dram_tensor("weight", [C, 1152], F32, kind="ExternalInput").ap()
    bias_d = nc.dram_tensor("bias", [C, 1], F32, kind="ExternalInput").ap()
    out_d = nc.dram_tensor("out", [C, HW], F32, kind="ExternalOutput").ap()
    xt2_d = nc.dram_tensor("xt2_pad", [GROWS, 256], BF16, kind="Internal").ap()
    consts_d = nc.dram_tensor("consts", [128, 707], F32, kind="ExternalInput").ap()

    ctx = ExitStack()
    with ctx:
        cpool = ctx.enter_context(tc.tile_pool(name="const", bufs=1))
        spool = ctx.enter_context(tc.tile_pool(name="setup", bufs=1))
        dpool = ctx.enter_context(tc.tile_pool(name="data", bufs=1))
        gpool = ctx.enter_context(tc.tile_pool(name="gath", bufs=4))
        vpool = ctx.enter_context(tc.tile_pool(name="val", bufs=2))
        ppool = ctx.enter_context(tc.tile_pool(name="psum", bufs=1, space="PSUM"))
        tpool = ctx.enter_context(tc.tile_pool(name="trps", bufs=2, space="PSUM"))
        opool = ctx.enter_context(tc.tile_pool(name="omps", bufs=2, space="PSUM"))

        # ---------- loads ----------
        cons = cpool.tile([128, 707], F32)
        nc.sync.dma_start(cons[:], consts_d[:, :])
        ident = cons[:, 0:128]
        hob = cons[:, 129:130]
        wo_r = cons[:, 130:131]
        ykc = cons[:, 131:419]
        xkc = cons[:, 419:707]

        x16 = spool.tile([128, HW], BF16)
        nc.gpsimd.dma_start(x16[:], x_d[:, :])
        w_om16 = spool.tile([27, 1152], BF16)
        nc.gpsimd.dma_start(w_om16[:], w_om_d[:, :])
        w16 = spool.tile([128, 1152], BF16)
        nc.gpsimd.dma_start(w16[:], weight_d[:, :])
        nc.gpsimd.load_library(library_config.mlp)

        b_om_sb = spool.tile([27, 1], F32)
        nc.sync.dma_start(b_om_sb[:], b_om_d[:, :])
        bias_sb = spool.tile([128, 1], F32)
        nc.sync.dma_start(bias_sb[:], bias_d[:, :])

        identb = spool.tile([128, 128], BF16)
        nc.vector.tensor_copy(identb[:], ident)

        # ---------- x_pad (bf16) for the offset conv ----------
        XP = 66
        x_pad = spool.tile([128, XP * XP], BF16)
        nc.vector.memset(x_pad[:], 0.0)
        nc.vector.tensor_copy(
            x_pad[:].rearrange("p (a b) -> p a b", a=XP)[:, 1:65, 1:65],
            x16[:].rearrange("p (a b) -> p a b", a=64),
        )

        # ---------- weight transposes ----------
        womT = spool.tile([128, K2, 27], BF16)
        for k in range(K2):
            trp = tpool.tile([128, 512], BF16, tag="tr", name="trp")
            nc.tensor.transpose(
                trp[:, 0:27],
                w_om16[:].rearrange("p (c k) -> p c k", k=K2)[:, :, k],
                identb[0:27, 0:27],
            )
            nc.scalar.copy(womT[:, k, :], trp[:, 0:27])

        # ---------- offset conv: om (27, 4096) bf16 ----------
        om_sb = spool.tile([27, HW], BF16)
        xpv = x_pad[:].rearrange("p (a b) -> p a b", a=XP)
        for ch in range(8):
            omp = opool.tile([128, 512], F32, tag="om", name="omp")
            for k in range(K2):
                dy_, dx_ = k // 3, k % 3
                r0 = ch * 8 + dy_
                nc.tensor.matmul(
                    omp[0:27, :], womT[:, k, :], xpv[:, r0:r0 + 8, dx_:dx_ + 64],
                    start=(k == 0), stop=(k == K2 - 1),
                )
            nc.scalar.activation(
                om_sb[:, ch * 512:(ch + 1) * 512], omp[0:27, :],
                IDENT, bias=b_om_sb[:], scale=1.0,
            )

        # ---------- omT (128 pix, 27) per s-tile; 4 tiles per psum copy ----
        omT = spool.tile([128, NS, 27], F32)
        for s4 in range(NS // 4):
            trp = tpool.tile([128, 512], BF16, tag="tr", name="trp")
            for j in range(4):
                nc.tensor.transpose(
                    trp[:, j * 128:j * 128 + 27],
                    om_sb[:, (4 * s4 + j) * 128:(4 * s4 + j + 1) * 128],
                    identb[0:27, 0:27],
                )
            nc.scalar.copy(
                omT[:, 4 * s4:4 * s4 + 4, :],
                trp[:].rearrange("p (a b) -> p a b", b=128)[:, :, 0:27],
            )

        # ---------- zero-fill xt2 on the scalar HWDGE ring ----------
        zt = spool.tile([128, 1296], BF16)
        nc.vector.memset(zt[:], 0.0)
        for i in range(8):
            nc.scalar.dma_start(
                AP(xt2_d.tensor, i * 128 * 1296, [[1296, 128], [1, 1296]]), zt[:]
            )

        # ---------- stage xt2 ----------
        # xt2 row (y+PADG, x) = [C(y,x), C(y+1,x)].  Per s we build the two
        # full rows y=2s,2s+1 as a [64, 512] tile (partition = x) so the DMA
        # writes are 512B-contiguous runs:
        #   stg2[x] = [C(2s,x) C(2s+1,x) | C(2s+1,x) C(2s+2,x)]
        stg0 = None
        for s in range(NS):
            trp = tpool.tile([128, 512], BF16, tag="tr", name="trp")
            nc.tensor.transpose(trp[0:64, 0:128],
                                x16[:, 2 * s * 64:(2 * s + 1) * 64], identb[:])
            nc.tensor.transpose(trp[0:64, 128:256],
                                x16[:, (2 * s + 1) * 64:(2 * s + 2) * 64],
                                identb[:])
            if s < NS - 1:
                nc.tensor.transpose(trp[0:64, 256:384],
                                    x16[:, (2 * s + 2) * 64:(2 * s + 3) * 64],
                                    identb[:])
            stg2 = vpool.tile([64, 512], BF16, tag="stg", name="stg")
            # cols [0:256]=[C(2s) C(2s+1)], [256:512]=[C(2s+1) C(2s+2)]:
            # overlapping read duplicates the middle block
            nc.vector.tensor_copy(
                stg2[:].rearrange("p (a b) -> p a b", a=2),
                AP(trp[:].tensor, trp[:].offset, [[512, 64], [128, 2], [1, 256]]),
            )
            if s == 0:
                stg0 = stg2
            eng = nc.sync if s % 2 == 0 else nc.scalar
            if s < NS - 1:
                eng.dma_start(
                    AP(xt2_d.tensor, ((2 * s + PADG) * GW + PADG) * 256,
                       [[256, 64], [GW * 256, 2], [1, 256]]),
                    AP(stg2[:].tensor, stg2[:].offset,
                       [[512, 64], [256, 2], [1, 256]]),
                )
            else:
                eng.dma_start(
                    AP(xt2_d.tensor, ((2 * s + PADG) * GW + PADG) * 256,
                       [[256, 64], [1, 256]]),
                    stg2[:, 0:256],
                )
                eng.dma_start(
                    AP(xt2_d.tensor, ((2 * s + 1 + PADG) * GW + PADG) * 256,
                       [[256, 64], [1, 128]]),
                    stg2[:, 256:384],
                )
        # row PADG-1 second half = C(0): pad row just below the image
        nc.scalar.dma_start(
            AP(xt2_d.tensor, ((PADG - 1) * GW + PADG) * 256 + 128,
               [[256, 64], [1, 128]]),
            stg0[:, 0:128],
        )

        # ---------- sampling positions -> gather indices (before mask) ----
        _cnt = [0]

        def f(shape=(128, NS, K2), dt=F32, tag=None):
            _cnt[0] += 1
            nm = f"cf{_cnt[0]}"
            return dpool.tile(list(shape), dt, tag=tag or nm, name=nm)

        omT_t = omT[:].tensor
        omT_off = omT[:].offset
        dyT = AP(omT_t, omT_off + 0, [[NS * 27, 128], [27, NS], [2, K2]])
        dxT = AP(omT_t, omT_off + 1, [[NS * 27, 128], [27, NS], [2, K2]])
        mlg = omT[:, :, 18:27]

        ykv = ykc.rearrange("p (s a) -> p s a", a=K2)
        xkv = xkc.rearrange("p (s a) -> p s a", a=K2)
        py = f()
        nc.vector.scalar_tensor_tensor(py[:], dyT, hob, ykv, Alu.add, Alu.add)
        px = f()
        nc.vector.scalar_tensor_tensor(px[:], dxT, wo_r, xkv, Alu.add, Alu.add)

        def floorit(v):
            vi = f(dt=I32, tag="fl_i")
            nc.vector.tensor_copy(vi[:], v[:])
            v0 = f(tag="fl_f")
            nc.vector.tensor_copy(v0[:], vi[:])
            gt = f(tag="fl_gt")
            nc.vector.tensor_tensor(gt[:], v0[:], v[:], Alu.is_gt)
            v0f = f()
            nc.vector.tensor_tensor(v0f[:], v0[:], gt[:], Alu.subtract)
            return v0f

        y0f = floorit(py)
        x0f = floorit(px)
        nc.vector.tensor_scalar(y0f[:], y0f[:], -float(PADG), float(H + 2),
                                Alu.max, Alu.min)
        nc.vector.tensor_scalar(x0f[:], x0f[:], -float(PADG), float(W + 2),
                                Alu.max, Alu.min)

        # row index r = (y0+PADG)*GW + (x0+PADG), written k-major: gKM[p][k][s]
        gAf = f()
        nc.vector.tensor_scalar(gAf[:], y0f[:], float(GW), float(PADG * GW + PADG),
                                Alu.mult, Alu.add)
        gKM = dpool.tile([128, K2, NS], F32, tag="gKM", name="gKM")
        gKM_w = AP(gKM[:].tensor, gKM[:].offset, [[K2 * NS, 128], [1, NS], [NS, K2]])
        nc.vector.tensor_tensor(gKM_w, gAf[:], x0f[:], Alu.add)

        # ---------- idx wrap via PE transposes ----------
        # want idxAw[16u'+pp][k*256 + s*8 + u] = gKM[16u+pp][k][s] for all u'
        idxAw = spool.tile([128, K2 * 256], I16)
        t1s = []
        for g in range(3):  # pass 1: [128, 96] -> [96, 128], 96 = 3 taps x 32 s
            trp = opool.tile([128, 512], F32, tag="om", name="omp")
            nc.tensor.transpose(
                trp[0:96, 0:128],
                gKM[:].rearrange("p a b -> p (a b)")[:, 96 * g:96 * (g + 1)],
                ident[:],
            )
            t1 = spool.tile([96, 128], F32, tag=f"t1_{g}")
            nc.scalar.copy(t1[:], trp[0:96, 0:128])
            t1s.append(t1)
        for g in range(3):
            for u4 in range(2):  # pass 2: 4x [96, 16] -> [16, 96] per psum buf
                trp = opool.tile([128, 512], F32, tag="om", name="omp")
                for j in range(4):
                    u = 4 * u4 + j
                    nc.tensor.transpose(
                        trp[0:16, j * 128:j * 128 + 96],
                        t1s[g][:, 16 * u:16 * u + 16],
                        ident[0:96, 0:96],
                    )
                t2 = vpool.tile([16, 512], F32, tag="t2", name="t2")
                nc.scalar.copy(t2[:], trp[0:16, :])
                # scatter (j, k', s) -> col (3g+k')*256 + s*8 + (4*u4+j)
                dst = AP(
                    idxAw[:].tensor,
                    idxAw[:].offset + (3 * g) * 256 + 4 * u4,
                    [[K2 * 256, 16], [1, 4], [256, 3], [8, NS]],
                )
                src = AP(
                    t2[:].tensor, t2[:].offset,
                    [[512, 16], [128, 4], [32, 3], [1, 32]],
                )
                nc.vector.tensor_copy(dst, src)
        for u in range(1, 8):  # replicate to all 8 16-partition groups
            nc.scalar.dma_start(idxAw[16 * u:16 * u + 16, :], idxAw[0:16, :])

        wT = spool.tile([128, K2, 128], BF16)
        for k in range(K2):
            trp = tpool.tile([128, 512], BF16, tag="tr", name="trp")
            nc.tensor.transpose(
                trp[:, 0:128],
                w16[:].rearrange("p (c k) -> p c k", k=K2)[:, :, k], identb[:],
            )
            nc.scalar.copy(wT[:, k, :], trp[:, 0:128])

        # ---------- softmax mask + bilinear coefficients (overlap gathers) --
        e = f()
        nc.scalar.activation(e[:], mlg, mybir.ActivationFunctionType.Exp)
        ssum = f((128, NS, 1))
        nc.vector.tensor_reduce(ssum[:], e[:], mybir.AxisListType.X, Alu.add)
        rs = f((128, NS, 1))
        nc.vector.reciprocal(rs[:], ssum[:])
        mask = f()
        nc.vector.tensor_tensor(mask[:], e[:], rs[:].to_broadcast([128, NS, K2]),
                                Alu.mult)

        wy1 = f()
        nc.vector.tensor_tensor(wy1[:], py[:], y0f[:], Alu.subtract)
        wy0 = f()
        nc.vector.tensor_scalar(wy0[:], wy1[:], -1.0, 1.0, Alu.mult, Alu.add)
        wx1 = f()
        nc.vector.tensor_tensor(wx1[:], px[:], x0f[:], Alu.subtract)
        wx0 = f()
        nc.vector.tensor_scalar(wx0[:], wx1[:], -1.0, 1.0, Alu.mult, Alu.add)

        mwy0 = f()
        nc.vector.tensor_tensor(mwy0[:], mask[:], wy0[:], Alu.mult)
        mwy1 = f()
        nc.vector.tensor_tensor(mwy1[:], mask[:], wy1[:], Alu.mult)
        c00 = f()
        nc.vector.tensor_tensor(c00[:], mwy0[:], wx0[:], Alu.mult)
        c01 = f()
        nc.vector.tensor_tensor(c01[:], mwy0[:], wx1[:], Alu.mult)
        c10 = f()
        nc.vector.tensor_tensor(c10[:], mwy1[:], wx0[:], Alu.mult)
        c11 = f()
        nc.vector.tensor_tensor(c11[:], mwy1[:], wx1[:], Alu.mult)

        # ---------- main loop ----------
        out_sb = spool.tile([128, HW], F32)
        xt2_src = AP(xt2_d.tensor, 0, [[256, GROWS - 1], [1, 512]])
        for h in range(NHALF):
            outp = ppool.tile([128, PPH], F32, tag="out", name="outp")
            for k in range(K2):
                # last tap split into two half-gathers to shorten the tail
                parts = ((0, SPH),) if k < K2 - 1 else ((0, 8), (8, SPH))
                for (t0, t1) in parts:
                    nt = t1 - t0
                    gb = gpool.tile([128, nt, 512], BF16, tag="gb", name="gb")
                    nc.gpsimd.dma_gather(
                        gb[:], xt2_src,
                        idxAw[:, k * 256 + 128 * h + 8 * t0:
                              k * 256 + 128 * h + 8 * t0 + 8 * nt],
                        128 * nt, 128 * nt, 512, elem_step=256,
                        single_packet=SINGLE_PACKET,
                        queue_num=(h * K2 + k + t0 // 8) % 4,
                    )
                    # corners: [0:128]=A0(c00) [128:256]=B0(c10)
                    #          [256:384]=A1(c01) [384:512]=B1(c11)
                    mb = vpool.tile([128, nt, 128], BF16, tag="mb", name="mb")
                    for t in range(nt):
                        s = h * SPH + t0 + t
                        nc.scalar.activation(mb[:, t, :], gb[:, t, 0:128], IDENT,
                                             bias=0.0, scale=c00[:, s, k:k + 1])
                    hs = h * SPH + t0
                    u1 = vpool.tile([128, nt, 128], BF16, tag="u1", name="u1")
                    nc.vector.tensor_tensor(
                        u1[:], gb[:, :, 256:384],
                        c01[:, hs:hs + nt, k:k + 1].to_broadcast([128, nt, 128]),
                        Alu.mult)
                    u2 = vpool.tile([128, nt, 128], BF16, tag="u2", name="u2")
                    nc.vector.tensor_tensor(
                        u2[:], gb[:, :, 128:256],
                        c10[:, hs:hs + nt, k:k + 1].to_broadcast([128, nt, 128]),
                        Alu.mult)
                    u3 = vpool.tile([128, nt, 128], BF16, tag="u3", name="u3")
                    nc.vector.tensor_tensor(
                        u3[:], gb[:, :, 384:512],
                        c11[:, hs:hs + nt, k:k + 1].to_broadcast([128, nt, 128]),
                        Alu.mult)
                    vb = vpool.tile([128, nt, 128], BF16, tag="vb", name="vb")
                    nc.vector.tensor_tensor(vb[:], u1[:], mb[:], Alu.add)
                    nc.vector.tensor_tensor(vb[:], vb[:], u2[:], Alu.add)
                    nc.vector.tensor_tensor(vb[:], vb[:], u3[:], Alu.add)

                    trp = None
                    for t in range(nt):
                        tg = t0 + t
                        if tg % 4 == 0:
                            trp = tpool.tile([128, 512], BF16, tag="tr",
                                             name="trp")
                        nc.tensor.transpose(
                            trp[:, (tg % 4) * 128:(tg % 4) * 128 + 128],
                            vb[:, t, :], identb[:])
                        if tg % 4 == 3:
                            vT = vpool.tile([128, 512], BF16, tag="vT",
                                            name="vT")
                            nc.scalar.copy(vT[:], trp[:])
                            bk = tg // 4
                            nc.tensor.matmul(
                                outp[:, bk * 512:(bk + 1) * 512], wT[:, k, :],
                                vT[:], start=(k == 0), stop=(k == K2 - 1),
                            )
            for bk in range(4):
                nc.scalar.activation(
                    out_sb[:, h * PPH + bk * 512: h * PPH + (bk + 1) * 512],
                    outp[:, bk * 512:(bk + 1) * 512],
                    IDENT, bias=bias_sb[:], scale=1.0,
                )
            nc.sync.dma_start(
                AP(out_d.tensor, h * PPH, [[HW, 128], [1, PPH]]),
                out_sb[:, h * PPH:(h + 1) * PPH],
            )


def _make_consts():
    c = np.zeros((128, 707), np.float32)
    c[:, 0:128] = np.eye(128, dtype=np.float32)
    p = np.arange(128)
    c[:, 128] = p
    c[:, 129] = (p >= 64)
    c[:, 130] = p % 64
    s = np.arange(32)[:, None, None]
    kyv = np.arange(3)[None, :, None]
    kxv = np.arange(3)[None, None, :]
    c[:, 131:419] = np.broadcast_to(
        (2 * s + kyv - 1 + 0 * kxv).reshape(-1), (128, 288))
    c[:, 419:707] = np.broadcast_to(
        (0 * s + 0 * kyv + kxv - 1).reshape(-1), (128, 288))
    return c


_COMPILED = None


def _get_compiled():
    global _COMPILED
    if _COMPILED is None:
        nc = bacc.Bacc(get_trn_type() or "TRN2", target_bir_lowering=False,
                       debug=False, num_devices=B, num_swdge_queues=4)
        with tile.TileContext(nc) as tc:
            _emit(tc)
        nc.compile()
        _COMPILED = nc
    return _COMPILED


def kernel(x, w_om, b_om, weight, bias):
    global LAST_EXEC_TIME_NS, LAST_RESULT
    x = np.ascontiguousarray(np.asarray(x, dtype=np.float32))
    w_om_f = np.ascontiguousarray(np.asarray(w_om, np.float32).reshape(27, 1152))
    b_om_f = np.ascontiguousarray(np.asarray(b_om, np.float32).reshape(27, 1))
    weight_f = np.ascontiguousarray(np.asarray(weight, np.float32).reshape(128, 1152))
    bias_f = np.ascontiguousarray(np.asarray(bias, np.float32).reshape(128, 1))

    nc = _get_compiled()
    consts = _make_consts()
    in_maps = [
        {
            "x": np.ascontiguousarray(x[b].reshape(C, HW)),
            "w_om": w_om_f,
            "b_om": b_om_f,
            "weight": weight_f,
            "bias": bias_f,
            "consts": consts,
        }
        for b in range(B)
    ]
    trace = bool(os.environ.get("DCN_TRACE"))
    res = run_bass_kernel_spmd(nc, in_maps, core_ids=list(range(B)), trace=trace)
    LAST_RESULT = res
    LAST_EXEC_TIME_NS = res.exec_time_ns
    out = np.stack([res.results[b]["out"].reshape(C, H, W) for b in range(B)])
    return out.astype(np.float32)


# revision 16
# speedup vs baseline: 1.0113x; 1.0113x over previous
"""DCNv2 (modulated deformable convolution) on 8 Trainium2 NeuronCores.

kernel(**inputs) takes the full unsharded inputs
    x      (8, 128, 64, 64) f32
    w_om   (27, 128, 3, 3)  f32
    b_om   (27,)            f32
    weight (128, 128, 3, 3) f32
    bias   (128,)           f32
and returns the full output (8, 128, 64, 64) f32.

Sharding: pure data-parallel over batch - one image per NeuronCore, small
weights replicated; no collectives.

v8 per-core program (bf16 datapath):
  1. x/weights are cast to bf16 during the load DMA (SWDGE); x is staged
     twice into a DRAM image xt2[GROWS, 256] where row r=(y,x) holds
     [C(y,x), C(y+1,x)] - the 4 bilinear corners of any sample are 4*128
     CONTIGUOUS bf16 values (one 1KB gather descriptor per (pixel, tap)).
     Row pairs are assembled on the PE and written as 512B-contiguous
     runs, split across the sync and scalar HWDGE rings.
  2. offset conv (27ch 3x3) on the PE in bf16; sampling positions + gather
     row indices are computed before the softmax mask / bilinear
     coefficients (which overlap the first gathers); indices are wrapped
     into the 16-partition dma_gather layout with PE transposes.
  3. per (half, tap) one dma_gather (SWDGE queues round-robined 0-3 so all
     four Q7 core pairs generate descriptors in parallel) fetches
     [A0 B0 A1 B1] corner blocks in (pixel-partition, channel) layout;
     corners are combined at whole-gather granularity: ACT applies c00 per
     pixel-tile (16 ops into one [128,16,128] tile), DVE does 3
     broadcast-coefficient multiplies + 3 bf16 adds; the result is
     PE-transposed back to (channel, pixel) (4 tiles per PSUM bank) and
     accumulated over the 9 taps into PSUM with the 128x128x3x3 weight;
     bias is added on the PSUM->SBUF copy.
"""

import os
import sys

import numpy as np

sys.path.insert(0, "/opt/trn_rl_repo")

from contextlib import ExitStack

import concourse.bacc as bacc
import concourse.mybir as mybir
import concourse.tile as tile
from concourse._compat import get_trn_type
from concourse.alu_op_type import AluOpType as Alu
from concourse.bass import AP
from concourse.bass_utils import run_bass_kernel_spmd
from concourse import library_config

F32 = mybir.dt.float32
BF16 = mybir.dt.bfloat16
I32 = mybir.dt.int32
I16 = mybir.dt.int16

B = 8
C = 128
H = W = 64
HW = H * W
K2 = 9
PADG = 4
GW = H + 2 * PADG      # 72
GROWS = GW * GW        # 5184
NS = 32
NHALF = 2
SPH = NS // NHALF      # 16 s-tiles per half
PPH = HW // NHALF      # 2048 pixels per half
IDENT = mybir.ActivationFunctionType.Identity

LAST_EXEC_TIME_NS = None
LAST_RESULT = None
SINGLE_PACKET = bool(int(os.environ.get("DCN_SP", "0")))


def _emit(tc):
    nc = tc.nc
    x_d = nc.dram_tensor("x", [C, HW], F32, kind="ExternalInput").ap()
    w_om_d = nc.dram_tensor("w_om", [27, 1152], F32, kind="ExternalInput").ap()
    b_om_d = nc.dram_tensor("b_om", [27, 1], F32, kind="ExternalInput").ap()
    weight_d = nc.dram_tensor("weight", [C, 1152], F32, kind="ExternalInput").ap()
    bias_d = nc.dram_tensor("bias", [C, 1], F32, kind="ExternalInput").ap()
    out_d = nc.dram_tensor("out", [C, HW], F32, kind="ExternalOutput").ap()
    xt2_d = nc.dram_tensor("xt2_pad", [GROWS, 256], BF16, kind="Internal").ap()
    consts_d = nc.dram_tensor("consts", [128, 707], F32, kind="ExternalInput").ap()

    ctx = ExitStack()
    with ctx:
        cpool = ctx.enter_context(tc.tile_pool(name="const", bufs=1))
        spool = ctx.enter_context(tc.tile_pool(name="setup", bufs=1))
        dpool = ctx.enter_context(tc.tile_pool(name="data", bufs=1))
        gpool = ctx.enter_context(tc.tile_pool(name="gath", bufs=4))
        vpool = ctx.enter_context(tc.tile_pool(name="val", bufs=2))
        ppool = ctx.enter_context(tc.tile_pool(name="psum", bufs=1, space="PSUM"))
        tpool = ctx.enter_context(tc.tile_pool(name="trps", bufs=2, space="PSUM"))
        opool = ctx.enter_context(tc.tile_pool(name="omps", bufs=2, space="PSUM"))

        # ---------- loads ----------
        cons = cpool.tile([128, 707], F32)
        nc.sync.dma_start(cons[:], consts_d[:, :])
        ident = cons[:, 0:128]
        hob = cons[:, 129:130]
        wo_r = cons[:, 130:131]
        ykc = cons[:, 131:419]
        xkc = cons[:, 419:707]

        x16 = spool.tile([128, HW], BF16)
        nc.gpsimd.dma_start(x16[:], x_d[:, :])
        w_om16 = spool.tile([27, 1152], BF16)
        nc.gpsimd.dma_start(w_om16[:], w_om_d[:, :])
        w16 = spool.tile([128, 1152], BF16)
        nc.gpsimd.dma_start(w16[:], weight_d[:, :])
        nc.gpsimd.load_library(library_config.mlp)

        b_om_sb = spool.tile([27, 1], F32)
        nc.sync.dma_start(b_om_sb[:], b_om_d[:, :])
        bias_sb = spool.tile([128, 1], F32)
        nc.sync.dma_start(bias_sb[:], bias_d[:, :])

        identb = spool.tile([128, 128], BF16)
        nc.vector.tensor_copy(identb[:], ident)

        # ---------- x_pad (bf16) for the offset conv ----------
        XP = 66
        x_pad = spool.tile([128, XP * XP], BF16)
        nc.vector.memset(x_pad[:], 0.0)
        nc.vector.tensor_copy(
            x_pad[:].rearrange("p (a b) -> p a b", a=XP)[:, 1:65, 1:65],
            x16[:].rearrange("p (a b) -> p a b", a=64),
        )

        # ---------- weight transposes ----------
        womT = spool.tile([128, K2, 27], BF16)
        for k in range(K2):
            trp = tpool.tile([128, 512], BF16, tag="tr", name="trp")
            nc.tensor.transpose(
                trp[:, 0:27],
                w_om16[:].rearrange("p (c k) -> p c k", k=K2)[:, :, k],
                identb[0:27, 0:27],
            )
            nc.scalar.copy(womT[:, k, :], trp[:, 0:27])

        # ---------- offset conv: om (27, 4096) bf16 ----------
        om_sb = spool.tile([27, HW], BF16)
        xpv = x_pad[:].rearrange("p (a b) -> p a b", a=XP)
        for ch in range(8):
            omp = opool.tile([128, 512], F32, tag="om", name="omp")
            for k in range(K2):
                dy_, dx_ = k // 3, k % 3
                r0 = ch * 8 + dy_
                nc.tensor.matmul(
                    omp[0:27, :], womT[:, k, :], xpv[:, r0:r0 + 8, dx_:dx_ + 64],
                    start=(k == 0), stop=(k == K2 - 1),
                )
            nc.scalar.activation(
                om_sb[:, ch * 512:(ch + 1) * 512], omp[0:27, :],
                IDENT, bias=b_om_sb[:], scale=1.0,
            )

        # ---------- omT (128 pix, 27) per s-tile; 4 tiles per psum copy ----
        omT = spool.tile([128, NS, 27], F32)
        for s4 in range(NS // 4):
            trp = tpool.tile([128, 512], BF16, tag="tr", name="trp")
            for j in range(4):
                nc.tensor.transpose(
                    trp[:, j * 128:j * 128 + 27],
                    om_sb[:, (4 * s4 + j) * 128:(4 * s4 + j + 1) * 128],
                    identb[0:27, 0:27],
                )
            nc.scalar.copy(
                omT[:, 4 * s4:4 * s4 + 4, :],
                trp[:].rearrange("p (a b) -> p a b", b=128)[:, :, 0:27],
            )

        # ---------- zero-fill xt2 on the scalar HWDGE ring ----------
        zt = spool.tile([128, 1296], BF16)
        nc.vector.memset(zt[:], 0.0)
        for i in range(8):
            nc.scalar.dma_start(
                AP(xt2_d.tensor, i * 128 * 1296, [[1296, 128], [1, 1296]]), zt[:]
            )

        # ---------- stage xt2 ----------
        # xt2 row (y+PADG, x) = [C(y,x), C(y+1,x)].  Per s we build the two
        # full rows y=2s,2s+1 as a [64, 512] tile (partition = x) so the DMA
        # writes are 512B-contiguous runs:
        #   stg2[x] = [C(2s,x) C(2s+1,x) | C(2s+1,x) C(2s+2,x)]
        stg0 = None
        for s in range(NS):
            trp = tpool.tile([128, 512], BF16, tag="tr", name="trp")
            nc.tensor.transpose(trp[0:64, 0:128],
                                x16[:, 2 * s * 64:(2 * s + 1) * 64], identb[:])
            nc.tensor.transpose(trp[0:64, 128:256],
                                x16[:, (2 * s + 1) * 64:(2 * s + 2) * 64],
                                identb[:])
            if s < NS - 1:
                nc.tensor.transpose(trp[0:64, 256:384],
                                    x16[:, (2 * s + 2) * 64:(2 * s + 3) * 64],
                                    identb[:])
            stg2 = vpool.tile([64, 512], BF16, tag="stg", name="stg")
            # cols [0:256]=[C(2s) C(2s+1)], [256:512]=[C(2s+1) C(2s+2)]:
            # overlapping read duplicates the middle block
            nc.vector.tensor_copy(
                stg2[:].rearrange("p (a b) -> p a b", a=2),
                AP(trp[:].tensor, trp[:].offset, [[512, 64], [128, 2], [1, 256]]),
            )
            if s == 0:
                stg0 = stg2
            eng = nc.sync if s % 2 == 0 else nc.scalar
            if s < NS - 1:
                eng.dma_start(
                    AP(xt2_d.tensor, ((2 * s + PADG) * GW + PADG) * 256,
                       [[256, 64], [GW * 256, 2], [1, 256]]),
                    AP(stg2[:].tensor, stg2[:].offset,
                       [[512, 64], [256, 2], [1, 256]]),
                )
            else:
                eng.dma_start(
                    AP(xt2_d.tensor, ((2 * s + PADG) * GW + PADG) * 256,
                       [[256, 64], [1, 256]]),
                    stg2[:, 0:256],
                )
                eng.dma_start(
                    AP(xt2_d.tensor, ((2 * s + 1 + PADG) * GW + PADG) * 256,
                       [[256, 64], [1, 128]]),
                    stg2[:, 256:384],
                )
        # row PADG-1 second half = C(0): pad row just below the image
        nc.scalar.dma_start(
            AP(xt2_d.tensor, ((PADG - 1) * GW + PADG) * 256 + 128,
               [[256, 64], [1, 128]]),
            stg0[:, 0:128],
        )

        # ---------- sampling positions -> gather indices (before mask) ----
        _cnt = [0]

        def f(shape=(128, NS, K2), dt=F32, tag=None):
            _cnt[0] += 1
            nm = f"cf{_cnt[0]}"
            return dpool.tile(list(shape), dt, tag=tag or nm, name=nm)

        omT_t = omT[:].tensor
        omT_off = omT[:].offset
        dyT = AP(omT_t, omT_off + 0, [[NS * 27, 128], [27, NS], [2, K2]])
        dxT = AP(omT_t, omT_off + 1, [[NS * 27, 128], [27, NS], [2, K2]])
        mlg = omT[:, :, 18:27]

        ykv = ykc.rearrange("p (s a) -> p s a", a=K2)
        xkv = xkc.rearrange("p (s a) -> p s a", a=K2)
        py = f()
        nc.vector.scalar_tensor_tensor(py[:], dyT, hob, ykv, Alu.add, Alu.add)
        px = f()
        nc.vector.scalar_tensor_tensor(px[:], dxT, wo_r, xkv, Alu.add, Alu.add)

        def floorit(v):
            vi = f(dt=I32, tag="fl_i")
            nc.vector.tensor_copy(vi[:], v[:])
            v0 = f(tag="fl_f")
            nc.vector.tensor_copy(v0[:], vi[:])
            gt = f(tag="fl_gt")
            nc.vector.tensor_tensor(gt[:], v0[:], v[:], Alu.is_gt)
            v0f = f()
            nc.vector.tensor_tensor(v0f[:], v0[:], gt[:], Alu.subtract)
            return v0f

        y0f = floorit(py)
        x0f = floorit(px)
        nc.vector.tensor_scalar(y0f[:], y0f[:], -float(PADG), float(H + 2),
                                Alu.max, Alu.min)
        nc.vector.tensor_scalar(x0f[:], x0f[:], -float(PADG), float(W + 2),
                                Alu.max, Alu.min)

        # row index r = (y0+PADG)*GW + (x0+PADG), written k-major: gKM[p][k][s]
        gAf = f()
        nc.vector.tensor_scalar(gAf[:], y0f[:], float(GW), float(PADG * GW + PADG),
                                Alu.mult, Alu.add)
        gKM = dpool.tile([128, K2, NS], F32, tag="gKM", name="gKM")
        gKM_w = AP(gKM[:].tensor, gKM[:].offset, [[K2 * NS, 128], [1, NS], [NS, K2]])
        nc.vector.tensor_tensor(gKM_w, gAf[:], x0f[:], Alu.add)

        # ---------- idx wrap via PE transposes ----------
        # want idxAw[16u'+pp][k*256 + s*8 + u] = gKM[16u+pp][k][s] for all u'
        idxAw = spool.tile([128, K2 * 256], I16)
        t1s = []
        for g in range(3):  # pass 1: [128, 96] -> [96, 128], 96 = 3 taps x 32 s
            trp = opool.tile([128, 512], F32, tag="om", name="omp")
            nc.tensor.transpose(
                trp[0:96, 0:128],
                gKM[:].rearrange("p a b -> p (a b)")[:, 96 * g:96 * (g + 1)],
                ident[:],
            )
            t1 = spool.tile([96, 128], F32, tag=f"t1_{g}")
            nc.scalar.copy(t1[:], trp[0:96, 0:128])
            t1s.append(t1)
        for g in range(3):
            for u4 in range(2):  # pass 2: 4x [96, 16] -> [16, 96] per psum buf
                trp = opool.tile([128, 512], F32, tag="om", name="omp")
                for j in range(4):
                    u = 4 * u4 + j
                    nc.tensor.transpose(
                        trp[0:16, j * 128:j * 128 + 96],
                        t1s[g][:, 16 * u:16 * u + 16],
                        ident[0:96, 0:96],
                    )
                t2 = vpool.tile([16, 512], F32, tag="t2", name="t2")
                nc.scalar.copy(t2[:], trp[0:16, :])
                # scatter (j, k', s) -> col (3g+k')*256 + s*8 + (4*u4+j)
                dst = AP(
                    idxAw[:].tensor,
                    idxAw[:].offset + (3 * g) * 256 + 4 * u4,
                    [[K2 * 256, 16], [1, 4], [256, 3], [8, NS]],
                )
                src = AP(
                    t2[:].tensor, t2[:].offset,
                    [[512, 16], [128, 4], [32, 3], [1, 32]],
                )
                nc.vector.tensor_copy(dst, src)
        for u in range(1, 8):  # replicate to all 8 16-partition groups
            nc.scalar.dma_start(idxAw[16 * u:16 * u + 16, :], idxAw[0:16, :])

        wT = spool.tile([128, K2, 128], BF16)
        for k in range(K2):
            trp = tpool.tile([128, 512], BF16, tag="tr", name="trp")
            nc.tensor.transpose(
                trp[:, 0:128],
                w16[:].rearrange("p (c k) -> p c k", k=K2)[:, :, k], identb[:],
            )
            nc.scalar.copy(wT[:, k, :], trp[:, 0:128])

        # ---------- softmax mask + bilinear coefficients (overlap gathers) --
        e = f()
        nc.scalar.activation(e[:], mlg, mybir.ActivationFunctionType.Exp)
        ssum = f((128, NS, 1))
        nc.vector.tensor_reduce(ssum[:], e[:], mybir.AxisListType.X, Alu.add)
        rs = f((128, NS, 1))
        nc.vector.reciprocal(rs[:], ssum[:])
        mask = f()
        nc.vector.tensor_tensor(mask[:], e[:], rs[:].to_broadcast([128, NS, K2]),
                                Alu.mult)

        wy1 = f()
        nc.vector.tensor_tensor(wy1[:], py[:], y0f[:], Alu.subtract)
        wy0 = f()
        nc.vector.tensor_scalar(wy0[:], wy1[:], -1.0, 1.0, Alu.mult, Alu.add)
        wx1 = f()
        nc.vector.tensor_tensor(wx1[:], px[:], x0f[:], Alu.subtract)
        wx0 = f()
        nc.vector.tensor_scalar(wx0[:], wx1[:], -1.0, 1.0, Alu.mult, Alu.add)

        mwy0 = f()
        nc.vector.tensor_tensor(mwy0[:], mask[:], wy0[:], Alu.mult)
        mwy1 = f()
        nc.vector.tensor_tensor(mwy1[:], mask[:], wy1[:], Alu.mult)
        c00 = f()
        nc.vector.tensor_tensor(c00[:], mwy0[:], wx0[:], Alu.mult)
        c01 = f()
        nc.vector.tensor_tensor(c01[:], mwy0[:], wx1[:], Alu.mult)
        c10 = f()
        nc.vector.tensor_tensor(c10[:], mwy1[:], wx0[:], Alu.mult)
        c11 = f()
        nc.vector.tensor_tensor(c11[:], mwy1[:], wx1[:], Alu.mult)

        # ---------- main loop ----------
        out_sb = spool.tile([128, HW], F32)
        xt2_src = AP(xt2_d.tensor, 0, [[256, GROWS - 1], [1, 512]])
        for h in range(NHALF):
            outp = ppool.tile([128, PPH], F32, tag="out", name="outp")
            for k in range(K2):
                # last tap split into two half-gathers to shorten the tail
                parts = ((0, SPH),) if k < K2 - 1 else ((0, 8), (8, SPH))
                for (t0, t1) in parts:
                    nt = t1 - t0
                    gb = gpool.tile([128, nt, 512], BF16, tag="gb", name="gb")
                    nc.gpsimd.dma_gather(
                        gb[:], xt2_src,
                        idxAw[:, k * 256 + 128 * h + 8 * t0:
                              k * 256 + 128 * h + 8 * t0 + 8 * nt],
                        128 * nt, 128 * nt, 512, elem_step=256,
                        single_packet=SINGLE_PACKET,
                        queue_num=(h * K2 + k + t0 // 8) % 4,
                    )
                    # corners: [0:128]=A0(c00) [128:256]=B0(c10)
                    #          [256:384]=A1(c01) [384:512]=B1(c11)
                    mb = vpool.tile([128, nt, 128], BF16, tag="mb", name="mb")
                    for t in range(nt):
                        s = h * SPH + t0 + t
                        nc.scalar.activation(mb[:, t, :], gb[:, t, 0:128], IDENT,
                                             bias=0.0, scale=c00[:, s, k:k + 1])
                    hs = h * SPH + t0
                    u1 = vpool.tile([128, nt, 128], BF16, tag="u1", name="u1")
                    nc.vector.tensor_tensor(
                        u1[:], gb[:, :, 256:384],
                        c01[:, hs:hs + nt, k:k + 1].to_broadcast([128, nt, 128]),
                        Alu.mult)
                    u2 = vpool.tile([128, nt, 128], BF16, tag="u2", name="u2")
                    nc.vector.tensor_tensor(
                        u2[:], gb[:, :, 128:256],
                        c10[:, hs:hs + nt, k:k + 1].to_broadcast([128, nt, 128]),
                        Alu.mult)
                    u3 = vpool.tile([128, nt, 128], BF16, tag="u3", name="u3")
                    nc.vector.tensor_tensor(
                        u3[:], gb[:, :, 384:512],
                        c11[:, hs:hs + nt, k:k + 1].to_broadcast([128, nt, 128]),
                        Alu.mult)
                    vb = vpool.tile([128, nt, 128], BF16, tag="vb", name="vb")
                    nc.vector.tensor_tensor(vb[:], u1[:], u2[:], Alu.add)
                    nc.vector.tensor_tensor(vb[:], vb[:], u3[:], Alu.add)
                    nc.vector.tensor_tensor(vb[:], vb[:], mb[:], Alu.add)

                    trp = None
                    for t in range(nt):
                        tg = t0 + t
                        if tg % 4 == 0:
                            trp = tpool.tile([128, 512], BF16, tag="tr",
                                             name="trp")
                        nc.tensor.transpose(
                            trp[:, (tg % 4) * 128:(tg % 4) * 128 + 128],
                            vb[:, t, :], identb[:])
                        if tg % 4 == 3:
                            vT = vpool.tile([128, 512], BF16, tag="vT",
                                            name="vT")
                            nc.scalar.copy(vT[:], trp[:])
                            bk = tg // 4
                            nc.tensor.matmul(
                                outp[:, bk * 512:(bk + 1) * 512], wT[:, k, :],
                                vT[:], start=(k == 0), stop=(k == K2 - 1),
                            )
            for bk in range(4):
                nc.scalar.activation(
                    out_sb[:, h * PPH + bk * 512: h * PPH + (bk + 1) * 512],
                    outp[:, bk * 512:(bk + 1) * 512],
                    IDENT, bias=bias_sb[:], scale=1.0,
                )
            nc.sync.dma_start(
                AP(out_d.tensor, h * PPH, [[HW, 128], [1, PPH]]),
                out_sb[:, h * PPH:(h + 1) * PPH],
            )


def _make_consts():
    c = np.zeros((128, 707), np.float32)
    c[:, 0:128] = np.eye(128, dtype=np.float32)
    p = np.arange(128)
    c[:, 128] = p
    c[:, 129] = (p >= 64)
    c[:, 130] = p % 64
    s = np.arange(32)[:, None, None]
    kyv = np.arange(3)[None, :, None]
    kxv = np.arange(3)[None, None, :]
    c[:, 131:419] = np.broadcast_to(
        (2 * s + kyv - 1 + 0 * kxv).reshape(-1), (128, 288))
    c[:, 419:707] = np.broadcast_to(
        (0 * s + 0 * kyv + kxv - 1).reshape(-1), (128, 288))
    return c


_COMPILED = None


def _get_compiled():
    global _COMPILED
    if _COMPILED is None:
        nc = bacc.Bacc(get_trn_type() or "TRN2", target_bir_lowering=False,
                       debug=False, num_devices=B, num_swdge_queues=4)
        with tile.TileContext(nc) as tc:
            _emit(tc)
        nc.compile()
        _COMPILED = nc
    return _COMPILED


def kernel(x, w_om, b_om, weight, bias):
    global LAST_EXEC_TIME_NS, LAST_RESULT
    x = np.ascontiguousarray(np.asarray(x, dtype=np.float32))
    w_om_f = np.ascontiguousarray(np.asarray(w_om, np.float32).reshape(27, 1152))
    b_om_f = np.ascontiguousarray(np.asarray(b_om, np.float32).reshape(27, 1))
    weight_f = np.ascontiguousarray(np.asarray(weight, np.float32).reshape(128, 1152))
    bias_f = np.ascontiguousarray(np.asarray(bias, np.float32).reshape(128, 1))

    nc = _get_compiled()
    consts = _make_consts()
    in_maps = [
        {
            "x": np.ascontiguousarray(x[b].reshape(C, HW)),
            "w_om": w_om_f,
            "b_om": b_om_f,
            "weight": weight_f,
            "bias": bias_f,
            "consts": consts,
        }
        for b in range(B)
    ]
    trace = bool(os.environ.get("DCN_TRACE"))
    res = run_bass_kernel_spmd(nc, in_maps, core_ids=list(range(B)), trace=trace)
    LAST_RESULT = res
    LAST_EXEC_TIME_NS = res.exec_time_ns
    out = np.stack([res.results[b]["out"].reshape(C, H, W) for b in range(B)])
    return out.astype(np.float32)
